# revision 76
# baseline (speedup 1.0000x reference)
"""Multi-head causal attention (B=2, S=2048, D=1024, H=16, d=64) on 8 trn2 cores.

Sharding: core c -> batch b=c//4, head-group hg=c%4 (4 heads, 256 of 1024 dims).
Each core computes its 4 heads' attention + its partial out-projection; host
sums the 4 partials per batch and adds the bias.

Attention uses the transposed S^T[k,q] layout throughout; softmax
normalization is deferred via a ones-column appended to V.

Perf-critical structure (found via perfetto traces):
- ALL matmuls run in full 128x128 array mode. Row-tiling (tile_position)
  alternation was observed to cap the PE clock at ~1.2GHz; score matmuls
  instead use per-head zero-padded K stationaries (head h's 64 dims at
  their native partitions, zeros elsewhere) so no mode switches occur.
- Bulk (fully-causal) ctx matmuls use fp8e4 DoubleRow (P and V in fp8,
  2 k-chunks per matmul, 0.5 cyc/row); measured error impact is nil
  because softmax numerator/denominator fp8 errors cancel. Diagonal
  tiles stay bf16 with a 128-col causal mask multiply.
- x^T is pre-transposed and tile-packed on the host so every device DMA
  is a contiguous copy (no xbar transpose mode; both hwdge rings run in
  parallel); x arrives in 4 seq-chunks and attn(qc) needs chunks <= qc.
- Projections / V tiles / out-projections are emitted as filler units
  interleaved into the ACT-bound attention stream; PE warmup matmuls
  ramp the clock while the first DMAs land. Output is bf16; host sums
  the 4 partial out-projections per batch and adds the bias.
"""
import sys

sys.path.insert(0, "/opt/trn_rl_repo")

import numpy as np
import ml_dtypes
import concourse.bass as bass
import concourse.mybir as mybir
from concourse import bacc
from concourse.tile import TileContext
from concourse.bass_utils import run_bass_kernel_spmd

F32 = mybir.dt.float32
BF16 = mybir.dt.bfloat16
FP8 = mybir.dt.float8e4
AF = mybir.ActivationFunctionType
OP = mybir.AluOpType
DR = mybir.MatmulPerfMode.DoubleRow

S = 2048          # sequence length
D = 1024          # model dim
HD = 64           # head dim
NHL = 4           # heads per core
DL = 256          # local out dims (NHL * HD)
NQC = 4           # q chunks of 512
QW = 512          # q chunk width
NKP = 16          # k chunks of 128
NST = 16          # seq tiles of 128
NIC = 8           # input-dim chunks of 128
HDP = 80          # padded fp8 V row (16B-aligned strides for DoubleRow)


def build_bass():
    nc = bacc.Bacc("TRN2", target_bir_lowering=False, debug=False, num_devices=8)

    xt0_d = nc.dram_tensor("xt0", [2, 128, NIC, QW // 2], BF16, kind="ExternalInput")
    xt_d = nc.dram_tensor("xt", [3, 128, NIC, QW], BF16, kind="ExternalInput")
    wq_d = nc.dram_tensor("wq", [128, NIC, DL], BF16, kind="ExternalInput")
    wk_d = nc.dram_tensor("wk", [128, NIC, DL], BF16, kind="ExternalInput")
    wv_d = nc.dram_tensor("wv", [128, NIC, DL], BF16, kind="ExternalInput")
    wo_d = nc.dram_tensor("wo", [128, 2, D], BF16, kind="ExternalInput")
    mb_d = nc.dram_tensor("maskb", [128, 2, QW], BF16, kind="ExternalInput")
    vo_d = nc.dram_tensor("vones", [128, NST * NHL], BF16, kind="ExternalInput")
    zz_d = nc.dram_tensor("zz", [128, S], BF16, kind="ExternalInput")
    out_d = nc.dram_tensor("out", [S, D], BF16, kind="ExternalOutput")

    with TileContext(nc) as tc:
        with (
            tc.tile_pool(name="consts", bufs=1) as consts,
            tc.tile_pool(name="xt", bufs=1) as xtp,
            tc.tile_pool(name="qk", bufs=1) as qkp,
            tc.tile_pool(name="vv", bufs=1) as vvp,
            tc.tile_pool(name="ctxn", bufs=1) as ctxnp,
            tc.tile_pool(name="ptp", bufs=4) as ptp,
            tc.tile_pool(name="rbp", bufs=2) as rbp,
            tc.tile_pool(name="outp", bufs=2) as outp,
            tc.tile_pool(name="stgp", bufs=8) as stgp,
            tc.tile_pool(name="psA", bufs=2, space="PSUM") as psA,
            tc.tile_pool(name="psC", bufs=2, space="PSUM") as psC,
            tc.tile_pool(name="psO", bufs=2, space="PSUM") as psO,
        ):
            # ---- input DMAs: weights on the ACT hwdge ring, x^T transposes
            # on the SP ring (4 chunks so projections start as chunks land).
            wq = consts.tile([128, NIC, DL], BF16, tag="wq")
            wk = consts.tile([128, NIC, DL], BF16, tag="wk")
            wv = consts.tile([128, NIC, DL], BF16, tag="wv")
            wo = consts.tile([128, 2, D], BF16, tag="wo")
            maskb = consts.tile([128, 2, QW], BF16, tag="maskb")
            vext = vvp.tile([128, NKP, NHL, HD + 1], BF16, tag="vext")
            # fp8 V copy (padded cols for 16B-aligned strides) for the bulk
            # DoubleRow ctx path; diag path stays bf16
            vext8 = vvp.tile([128, NKP // 2, 2, NHL, HDP], FP8, tag="vext8")

            # x^T is transposed on the host, so every DMA here is a plain
            # copy (no xbar transpose mode, both hwdge rings run parallel).
            # Chunk r holds all 8 input-dim chunks for seq block r — the
            # causal structure means attn(qc) only needs chunks <= qc.
            xt0a = xtp.tile([128, NIC, QW // 2], BF16, tag="xt0a", name="xt0a")
            xt0b = xtp.tile([128, NIC, QW // 2], BF16, tag="xt0b", name="xt0b")
            xtr = [None]
            for r in range(1, 4):
                t = xtp.tile([128, NIC, QW], BF16, tag=f"xt{r}", name=f"xt{r}")
                xtr.append(t)

            # ones column staged via a contiguous DMA + one strided DVE copy
            # (a direct DMA scatter of 2B elements poisons the hwdge ring)
            vstage = consts.tile([128, NST * NHL], BF16, tag="vstage")
            qt = qkp.tile([128, 2, S], BF16, tag="qt")
            # K stationaries stored zero-padded per head (head h data at its
            # native partitions, zeros elsewhere) so score matmuls run in
            # full 128x128 array mode -- no row-tiling mode switches.
            kt = qkp.tile([128, 2, 2, S], BF16, tag="kt")
            nc.scalar.dma_start(out=vstage, in_=vo_d.ap())
            nc.scalar.dma_start(out=xt0a, in_=xt0_d.ap()[0])
            nc.scalar.dma_start(out=xt0b, in_=xt0_d.ap()[1])
            nc.sync.dma_start(out=wq, in_=wq_d.ap())
            nc.sync.dma_start(out=wk, in_=wk_d.ap())
            for p in range(2):
                nc.sync.dma_start(out=kt[64:128, p, 0, :], in_=zz_d.ap()[64:128, :])
                nc.sync.dma_start(out=kt[0:64, p, 1, :], in_=zz_d.ap()[0:64, :])
            nc.vector.tensor_copy(
                vext[:, :, :, HD : HD + 1],
                vstage.rearrange("p (a b c) -> p a b c", a=NST, b=NHL),
            )
            nc.vector.tensor_copy(
                vext8[:, :, :, :, HD : HD + 1],
                vstage.rearrange("p (a b c d) -> p a b c d", a=NKP // 2, b=2, c=NHL),
            )
            nc.scalar.dma_start(out=xtr[1], in_=xt_d.ap()[0])
            nc.sync.dma_start(out=maskb, in_=mb_d.ap())
            nc.sync.dma_start(out=wv, in_=wv_d.ap())
            nc.sync.dma_start(out=xtr[2], in_=xt_d.ap()[1])
            nc.sync.dma_start(out=xtr[3], in_=xt_d.ap()[2])
            nc.sync.dma_start(out=wo, in_=wo_d.ap())

            def xts(ic, qc):
                return xtr[qc][:, ic, :]

            ctxn = ctxnp.tile([128, 2, S], BF16, tag="ctxn")

            # ---- PE warmup: dummy matmuls on a zeroed tile while the first
            # x/w DMAs are in flight, so the tensor engine's clock is ramped
            # when real work arrives.
            wtmp = consts.tile([128, QW], BF16, tag="wtmp")
            nc.vector.memset(wtmp, 0.0)
            wacc = psA.tile([128, 2, QW], F32, tag="st", name="wacc")
            for i in range(16):
                nc.tensor.matmul(
                    wacc[:, 0, 0:192],
                    wtmp[:, 0:128],
                    wtmp[:, 0:192],
                    start=(i == 0),
                    stop=False,
                )
            for i in range(22):
                nc.tensor.matmul(
                    wacc[:, 0, :],
                    wtmp[:, 0:128],
                    wtmp,
                    start=False,
                    stop=(i == 21),
                )

            # ---- pass1a: qt/kt for qc=0, both pairs, ic-outer (DMA-paced)
            accs = []
            for p in range(2):
                accs.append(psA.tile([128, 2, QW], F32, tag="st", name=f"acc{p}"))
            for half, xh in ((0, xt0a), (1, xt0b)):
                hs = slice(half * (QW // 2), (half + 1) * (QW // 2))
                for ic in range(NIC):
                    for p in range(2):
                        for lane, w in ((0, wq), (1, wk)):
                            nc.tensor.matmul(
                                accs[p][:, lane, hs],
                                w[:, ic, 128 * p : 128 * (p + 1)],
                                xh[:, ic, :],
                                start=(ic == 0),
                                stop=(ic == NIC - 1),
                            )
            for p in range(2):
                nc.any.tensor_copy(qt[:, p, 0:QW], accs[p][:, 0, :])
                nc.any.tensor_copy(kt[0:64, p, 0, 0:QW], accs[p][0:64, 1, :])
                nc.any.tensor_copy(kt[64:128, p, 1, 0:QW], accs[p][64:128, 1, :])

            # ---- filler units (each: PE work from the psO pool + evac)
            def qk_units(dst, w, p, qc):
                # two half-units sharing one accumulator for finer interleave
                hold = {}

                def fa():
                    hold["acc"] = psO.tile([128, QW], F32, tag="po", name="accqk")
                    for ic in range(NIC // 2):
                        nc.tensor.matmul(
                            hold["acc"],
                            w[:, ic, 128 * p : 128 * (p + 1)],
                            xts(ic, qc),
                            start=(ic == 0),
                            stop=False,
                        )

                def fb():
                    acc = hold["acc"]
                    for ic in range(NIC // 2, NIC):
                        nc.tensor.matmul(
                            acc,
                            w[:, ic, 128 * p : 128 * (p + 1)],
                            xts(ic, qc),
                            start=False,
                            stop=(ic == NIC - 1),
                        )
                    qsl = slice(qc * QW, (qc + 1) * QW)
                    if dst is qt:
                        nc.any.tensor_copy(dst[:, p, qsl], acc)
                    else:
                        nc.any.tensor_copy(dst[0:64, p, 0, qsl], acc[0:64, :])
                        nc.any.tensor_copy(dst[64:128, p, 1, qsl], acc[64:128, :])

                return [fa, fb]

            def v_unit(st):
                def f():
                    acc = psO.tile([128, 256], F32, tag="po", name="accv")
                    if st < 4:
                        xh = xt0a if st < 2 else xt0b
                        xs = (st % 2) * 128
                    else:
                        xh = xtr[st // 4]
                        xs = (st % 4) * 128
                    for ic in range(NIC):
                        nc.tensor.matmul(
                            acc,
                            xh[:, ic, xs : xs + 128],
                            wv[:, ic, :],
                            start=(ic == 0),
                            stop=(ic == NIC - 1),
                        )
                    nc.vector.tensor_copy(
                        vext[:, st, :, 0:HD],
                        acc.rearrange("p (h e) -> p h e", h=NHL),
                    )
                    if st < 12:  # st 12-15 are never below the diagonal
                        nc.vector.tensor_copy(
                            vext8[:, st // 2, st % 2, :, 0:HD],
                            acc.rearrange("p (h e) -> p h e", h=NHL),
                        )
                return f

            osb_hold = {}

            def o_unit(t, nh):
                def f():
                    tsl = slice(t * 128, (t + 1) * 128)
                    if nh == 0:
                        osb_hold[t] = outp.tile([128, D], BF16, tag="osb", name="osb")
                    osb = osb_hold[t]
                    po = psO.tile([128, QW], F32, tag="po", name="po")
                    nsl = slice(nh * QW, (nh + 1) * QW)
                    nc.tensor.matmul(
                        po, ctxn[:, 0, tsl], wo[:, 0, nsl], start=True, stop=False
                    )
                    nc.tensor.matmul(
                        po, ctxn[:, 1, tsl], wo[:, 1, nsl], start=False, stop=True
                    )
                    nc.any.tensor_copy(osb[:, nsl], po)
                    if nh == 1:
                        nc.sync.dma_start(out=out_d.ap()[tsl, :], in_=osb)
                return f

            # pair-split out-proj for the last q-chunk: the pair-0 half only
            # needs norm(3,0), so it can fill the ACT-bound attn(3,1) stretch
            stg_hold = {}

            def o_pre(t, nh):
                def f():
                    tsl = slice(t * 128, (t + 1) * 128)
                    po = psO.tile([128, QW], F32, tag="po", name="pop")
                    nsl = slice(nh * QW, (nh + 1) * QW)
                    nc.tensor.matmul(
                        po, ctxn[:, 0, tsl], wo[:, 0, nsl], start=True, stop=True
                    )
                    stg = stgp.tile([128, QW], F32, tag="stg", name="stg")
                    nc.any.tensor_copy(stg, po)
                    stg_hold[(t, nh)] = stg
                return f

            def o_post(t, nh):
                def f():
                    tsl = slice(t * 128, (t + 1) * 128)
                    if nh == 0:
                        osb_hold[t] = outp.tile([128, D], BF16, tag="osb", name="osb")
                    osb = osb_hold[t]
                    po = psO.tile([128, QW], F32, tag="po", name="po")
                    nsl = slice(nh * QW, (nh + 1) * QW)
                    nc.tensor.matmul(
                        po, ctxn[:, 1, tsl], wo[:, 1, nsl], start=True, stop=True
                    )
                    nc.vector.scalar_tensor_tensor(
                        out=osb[:, nsl],
                        in0=po,
                        scalar=1.0,
                        in1=stg_hold[(t, nh)],
                        op0=OP.mult,
                        op1=OP.add,
                    )
                    if nh == 1:
                        nc.sync.dma_start(out=out_d.ap()[tsl, :], in_=osb)
                return f

            Q = []

            def take():
                if Q:
                    Q.pop(0)()

            # ---- attention for one (q-chunk, head-pair): per-kp score tiles
            # with both heads in lanes (row-tiled 64+64), depth-1 ctx pipeline.
            def attn(qc, p):
                qsl = slice(qc * QW, (qc + 1) * QW)
                nkp = 4 * qc + 4
                ctxa = psC.tile([HDP, QW], F32, tag="ctx")
                ctxb = psC.tile([HDP, QW], F32, tag="ctx")
                take()
                take()
                pend = None
                # bulk kp pairs: fp8 DoubleRow ctx (2 k-chunks per matmul)
                for g in range(2 * qc):
                    sta = psA.tile([128, 2, QW], F32, tag="st", name="sta")
                    stb = psA.tile([128, 2, QW], F32, tag="st", name="stb")
                    for kig in range(2):
                        ksl = slice((2 * g + kig) * 128, (2 * g + kig + 1) * 128)
                        for st_t, h in ((sta, 0), (stb, 1)):
                            nc.tensor.matmul(
                                st_t[:, kig, :],
                                kt[:, p, h, ksl],
                                qt[:, p, qsl],
                                start=True,
                                stop=True,
                            )
                    pta = ptp.tile([128, 2, QW], FP8, tag="pt8", name="pta")
                    ptb = ptp.tile([128, 2, QW], FP8, tag="pt8", name="ptb")
                    nc.scalar.activation(pta, sta, AF.Exp, scale=0.125)
                    nc.scalar.activation(ptb, stb, AF.Exp, scale=0.125)
                    if pend is not None:
                        pend()
                    def mkb(g=g, pta=pta, ptb=ptb):
                        def c():
                            for h, ctx_t, pt_t in ((0, ctxa, pta), (1, ctxb, ptb)):
                                nc.tensor.matmul(
                                    ctx_t,
                                    vext8[:, g, :, 2 * p + h, :],
                                    pt_t,
                                    start=(g == 0),
                                    stop=False,
                                    perf_mode=DR,
                                )
                        return c
                    pend = mkb()
                    take()
                    take()
                # diagonal kp chunks: bf16, causally trimmed + masked
                for j in range(4):
                    kp = 4 * qc + j
                    w = QW - 128 * j
                    off = 128 * j
                    ksl = slice(kp * 128, (kp + 1) * 128)
                    qtr = slice(qc * QW + off, (qc + 1) * QW)
                    st = psA.tile([128, 2, QW], F32, tag="st", name="st")
                    pt = ptp.tile([128, 2, QW], BF16, tag="pt", name="pt")
                    for h in range(2):
                        nc.tensor.matmul(
                            st[:, h, 0:w],
                            kt[:, p, h, ksl],
                            qt[:, p, qtr],
                            start=True,
                            stop=True,
                        )
                    nc.scalar.activation(
                        pt[:, :, 0:w], st[:, :, 0:w], AF.Exp, scale=0.125
                    )
                    nc.vector.tensor_mul(
                        pt[:, :, 0:128], pt[:, :, 0:128], maskb[:, :, 0:128]
                    )
                    if pend is not None:
                        pend()
                    def mk(kp=kp, w=w, off=off, pt=pt):
                        def c():
                            for h, ctx_t in ((0, ctxa), (1, ctxb)):
                                nc.tensor.matmul(
                                    ctx_t[0 : HD + 1, off:QW],
                                    vext[:, kp, 2 * p + h, :],
                                    pt[:, h, 0:w],
                                    start=(kp == 0),
                                    stop=(kp == nkp - 1),
                                )
                        return c
                    pend = mk()
                    take()
                pend()
                take()
                take()
                # normalize: ctxn[d, q] = ctx[d, q] * (1 / ctx[64, q])
                for ctx_t, lo in ((ctxa, 0), (ctxb, 64)):
                    dcp = rbp.tile([1, QW], F32, tag="dcp")
                    nc.vector.tensor_copy(dcp, ctx_t[HD : HD + 1, :])
                    rec = rbp.tile([1, QW], F32, tag="rec")
                    nc.vector.reciprocal_approx_fast(rec, dcp)
                    rb = rbp.tile([HD, QW], F32, tag="rb")
                    nc.gpsimd.partition_broadcast(rb, rec)
                    # two halves so the first out-proj tiles unblock earlier
                    for hh in range(2):
                        hsl = slice(hh * 256, (hh + 1) * 256)
                        nc.vector.scalar_tensor_tensor(
                            out=ctxn[lo : lo + HD, p, qc * QW + hh * 256 :
                                     qc * QW + (hh + 1) * 256],
                            in0=ctx_t[0:HD, hsl],
                            scalar=1.0,
                            in1=rb[:, hsl],
                            op0=OP.mult,
                            op1=OP.mult,
                        )

            # ---- schedule
            for st in range(4):
                v_unit(st)()
            for p in range(2):
                Q += qk_units(qt, wq, p, 1)
                Q += qk_units(kt, wk, p, 1)
            attn(0, 0)
            Q += [v_unit(st) for st in range(4, 8)]
            attn(0, 1)
            for p in range(2):
                Q += qk_units(qt, wq, p, 2)
                Q += qk_units(kt, wk, p, 2)
            Q += [o_unit(t, nh) for t in range(0, 3) for nh in range(2)]
            attn(1, 0)
            Q += [v_unit(st) for st in range(8, 12)]
            Q += [o_unit(3, nh) for nh in range(2)]
            attn(1, 1)
            for p in range(2):
                Q += qk_units(qt, wq, p, 3)
                Q += qk_units(kt, wk, p, 3)
            Q += [o_unit(t, nh) for t in range(4, 7) for nh in range(2)]
            attn(2, 0)
            Q += [v_unit(st) for st in range(12, 16)]
            Q += [o_unit(7, nh) for nh in range(2)]
            attn(2, 1)
            Q += [o_unit(t, nh) for t in range(8, 10) for nh in range(2)]
            attn(3, 0)
            Q += [o_pre(t, nh) for t in range(12, 16) for nh in range(2)]
            attn(3, 1)
            # reserved units: fill the PE while norm(3,1) runs on DVE/GpSimd
            for t in range(10, 12):
                for nh in range(2):
                    o_unit(t, nh)()
            while Q:
                take()
            for t in range(12, 16):
                for nh in range(2):
                    o_post(t, nh)()

    nc.finalize()
    return nc


_VONES = np.ones((128, NST * NHL), dtype=ml_dtypes.bfloat16)
_ZZ = np.zeros((128, S), dtype=ml_dtypes.bfloat16)


def _maskb():
    # multiplicative causal mask: 0 where q_local < kp_local, else 1
    m = np.ones((128, QW), dtype=np.float32)
    kp = np.arange(128)[:, None]
    q = np.arange(QW)[None, :]
    m[q < kp] = 0.0
    return np.repeat(m[:, None, :], 2, axis=1).astype(ml_dtypes.bfloat16)


def _pack_w(w):
    # [D, dl] -> [128, D//128, dl] with (p, c, n) = w[128c+p, n]
    d, dl = w.shape
    return np.ascontiguousarray(w.reshape(d // 128, 128, dl).transpose(1, 0, 2))


def shard_inputs(x, Wq, Wk, Wv, Wo):
    x = np.asarray(x, dtype=ml_dtypes.bfloat16)
    # xt[b][r, p, c, s] = x[b, 512r+s, 128c+p]  (matches SBUF tile layout);
    # the first 512 rows are packed as two contiguous 256-row half blocks
    # so the first DMAs land (and projections start) as early as possible.
    xt0s = [
        np.ascontiguousarray(
            x[b][0:QW].reshape(2, QW // 2, NIC, 128).transpose(0, 3, 2, 1)
        )
        for b in range(2)
    ]
    xts = [
        np.ascontiguousarray(
            x[b][QW:].reshape(3, QW, NIC, 128).transpose(0, 3, 2, 1)
        )
        for b in range(2)
    ]
    Wq = np.asarray(Wq, dtype=ml_dtypes.bfloat16)
    Wk = np.asarray(Wk, dtype=ml_dtypes.bfloat16)
    Wv = np.asarray(Wv, dtype=ml_dtypes.bfloat16)
    Wo = np.asarray(Wo, dtype=ml_dtypes.bfloat16)
    mb = _maskb()
    in_maps = []
    for c in range(8):
        b, hg = divmod(c, 4)
        sl = slice(DL * hg, DL * (hg + 1))
        in_maps.append({
            "xt0": xt0s[b],
            "xt": xts[b],
            "wq": _pack_w(Wq[:, sl]),
            "wk": _pack_w(Wk[:, sl]),
            "wv": _pack_w(Wv[:, sl]),
            "wo": _pack_w(Wo[sl, :]),
            "maskb": mb,
            "vones": _VONES,
            "zz": _ZZ,
        })
    return in_maps


def run(inputs, trace=False, **kwargs):
    """Build, run on 8 cores, and return (full_output, BassKernelResults)."""
    nc = build_bass()
    bo = np.asarray(inputs["bo"], dtype=np.float32)
    in_maps = shard_inputs(**{k: v for k, v in inputs.items() if k != "bo"})
    res = run_bass_kernel_spmd(
        nc, in_maps, core_ids=list(range(8)), trace=trace, **kwargs
    )
    parts = [np.asarray(r["out"], dtype=np.float32) for r in res.results]
    out = np.empty((2, S, D), dtype=np.float32)
    for b in range(2):
        out[b] = parts[4 * b] + parts[4 * b + 1] + parts[4 * b + 2] + parts[4 * b + 3]
        out[b] += bo[None, :]
    return out, res


def kernel(x, Wq, Wk, Wv, Wo, bo):
    out, _ = run(dict(x=x, Wq=Wq, Wk=Wk, Wv=Wv, Wo=Wo, bo=bo))
    return out


# revision 77
# speedup vs baseline: 1.0429x; 1.0429x over previous
"""Multi-head causal attention (B=2, S=2048, D=1024, H=16, d=64) on 8 trn2 cores.

Sharding: core c -> batch b=c//4, head-group hg=c%4 (4 heads, 256 of 1024 dims).
Each core computes its 4 heads' attention + its partial out-projection; host
sums the 4 partials per batch and adds the bias.

Attention uses the transposed S^T[k,q] layout throughout; softmax
normalization is deferred via a ones-column appended to V.

Perf-critical structure (found via perfetto traces):
- ALL matmuls run in full 128x128 array mode. Row-tiling (tile_position)
  alternation was observed to cap the PE clock at ~1.2GHz; score matmuls
  instead use per-head zero-padded K stationaries (head h's 64 dims at
  their native partitions, zeros elsewhere) so no mode switches occur.
- Bulk (fully-causal) ctx matmuls use fp8e4 DoubleRow (P and V in fp8,
  2 k-chunks per matmul, 0.5 cyc/row); measured error impact is nil
  because softmax numerator/denominator fp8 errors cancel. Diagonal
  tiles stay bf16 with a 128-col causal mask multiply.
- x^T is pre-transposed and tile-packed on the host so every device DMA
  is a contiguous copy (no xbar transpose mode; both hwdge rings run in
  parallel); x arrives in 4 seq-chunks and attn(qc) needs chunks <= qc.
- Projections / V tiles / out-projections are emitted as filler units
  interleaved into the ACT-bound attention stream; PE warmup matmuls
  ramp the clock while the first DMAs land. Output is bf16; host sums
  the 4 partial out-projections per batch and adds the bias.
"""
import sys

sys.path.insert(0, "/opt/trn_rl_repo")

import numpy as np
import ml_dtypes
import concourse.bass as bass
import concourse.mybir as mybir
from concourse import bacc
from concourse.tile import TileContext
from concourse.bass_utils import run_bass_kernel_spmd

F32 = mybir.dt.float32
BF16 = mybir.dt.bfloat16
FP8 = mybir.dt.float8e4
AF = mybir.ActivationFunctionType
OP = mybir.AluOpType
DR = mybir.MatmulPerfMode.DoubleRow

S = 2048          # sequence length
D = 1024          # model dim
HD = 64           # head dim
NHL = 4           # heads per core
DL = 256          # local out dims (NHL * HD)
NQC = 4           # q chunks of 512
QW = 512          # q chunk width
NKP = 16          # k chunks of 128
NST = 16          # seq tiles of 128
NIC = 8           # input-dim chunks of 128
HDP = 80          # padded fp8 V row (16B-aligned strides for DoubleRow)


def build_bass():
    nc = bacc.Bacc("TRN2", target_bir_lowering=False, debug=False, num_devices=8)

    xt0_d = nc.dram_tensor("xt0", [2, 128, NIC, QW // 2], BF16, kind="ExternalInput")
    xt_d = nc.dram_tensor("xt", [3, 128, NIC, QW], BF16, kind="ExternalInput")
    wq_d = nc.dram_tensor("wq", [128, NIC, DL], BF16, kind="ExternalInput")
    wk_d = nc.dram_tensor("wk", [128, NIC, DL], BF16, kind="ExternalInput")
    wv_d = nc.dram_tensor("wv", [128, NIC, DL], BF16, kind="ExternalInput")
    wo_d = nc.dram_tensor("wo", [128, 2, D], BF16, kind="ExternalInput")
    mb_d = nc.dram_tensor("maskb", [128, 2, QW], BF16, kind="ExternalInput")
    vo_d = nc.dram_tensor("vones", [128, NST * NHL], BF16, kind="ExternalInput")
    zz_d = nc.dram_tensor("zz", [128, S], BF16, kind="ExternalInput")
    out_d = nc.dram_tensor("out", [S, D], BF16, kind="ExternalOutput")

    with TileContext(nc) as tc:
        with (
            tc.tile_pool(name="consts", bufs=1) as consts,
            tc.tile_pool(name="xt", bufs=1) as xtp,
            tc.tile_pool(name="qk", bufs=1) as qkp,
            tc.tile_pool(name="vv", bufs=1) as vvp,
            tc.tile_pool(name="ctxn", bufs=1) as ctxnp,
            tc.tile_pool(name="ptp", bufs=4) as ptp,
            tc.tile_pool(name="rbp", bufs=2) as rbp,
            tc.tile_pool(name="outp", bufs=2) as outp,
            tc.tile_pool(name="stgp", bufs=8) as stgp,
            tc.tile_pool(name="psA", bufs=2, space="PSUM") as psA,
            tc.tile_pool(name="psC", bufs=2, space="PSUM") as psC,
            tc.tile_pool(name="psO", bufs=2, space="PSUM") as psO,
        ):
            # ---- input DMAs: weights on the ACT hwdge ring, x^T transposes
            # on the SP ring (4 chunks so projections start as chunks land).
            wq = consts.tile([128, NIC, DL], BF16, tag="wq")
            wk = consts.tile([128, NIC, DL], BF16, tag="wk")
            wv = consts.tile([128, NIC, DL], BF16, tag="wv")
            wo = consts.tile([128, 2, D], BF16, tag="wo")
            maskb = consts.tile([128, 2, QW], BF16, tag="maskb")
            vext = vvp.tile([128, NKP, NHL, HD + 1], BF16, tag="vext")
            # fp8 V copy (padded cols for 16B-aligned strides) for the bulk
            # DoubleRow ctx path; diag path stays bf16
            vext8 = vvp.tile([128, NKP // 2, 2, NHL, HDP], FP8, tag="vext8")

            # x^T is transposed on the host, so every DMA here is a plain
            # copy (no xbar transpose mode, both hwdge rings run parallel).
            # Chunk r holds all 8 input-dim chunks for seq block r — the
            # causal structure means attn(qc) only needs chunks <= qc.
            xt0a = xtp.tile([128, NIC, QW // 2], BF16, tag="xt0a", name="xt0a")
            xt0b = xtp.tile([128, NIC, QW // 2], BF16, tag="xt0b", name="xt0b")
            xtr = [None]
            for r in range(1, 4):
                t = xtp.tile([128, NIC, QW], BF16, tag=f"xt{r}", name=f"xt{r}")
                xtr.append(t)

            # ones column staged via a contiguous DMA + one strided DVE copy
            # (a direct DMA scatter of 2B elements poisons the hwdge ring)
            vstage = consts.tile([128, NST * NHL], BF16, tag="vstage")
            qt = qkp.tile([128, 2, S], BF16, tag="qt")
            # K stationaries stored zero-padded per head (head h data at its
            # native partitions, zeros elsewhere) so score matmuls run in
            # full 128x128 array mode -- no row-tiling mode switches.
            kt = qkp.tile([128, 2, 2, S], BF16, tag="kt")
            nc.scalar.dma_start(out=vstage, in_=vo_d.ap())
            nc.scalar.dma_start(out=xt0a, in_=xt0_d.ap()[0])
            nc.scalar.dma_start(out=xt0b, in_=xt0_d.ap()[1])
            nc.sync.dma_start(out=wq, in_=wq_d.ap())
            nc.sync.dma_start(out=wk, in_=wk_d.ap())
            for p in range(2):
                nc.sync.dma_start(out=kt[64:128, p, 0, :], in_=zz_d.ap()[64:128, :])
                nc.sync.dma_start(out=kt[0:64, p, 1, :], in_=zz_d.ap()[0:64, :])
            nc.vector.tensor_copy(
                vext[:, :, :, HD : HD + 1],
                vstage.rearrange("p (a b c) -> p a b c", a=NST, b=NHL),
            )
            nc.vector.tensor_copy(
                vext8[:, :, :, :, HD : HD + 1],
                vstage.rearrange("p (a b c d) -> p a b c d", a=NKP // 2, b=2, c=NHL),
            )
            nc.scalar.dma_start(out=xtr[1], in_=xt_d.ap()[0])
            nc.sync.dma_start(out=maskb, in_=mb_d.ap())
            nc.sync.dma_start(out=wv, in_=wv_d.ap())
            nc.sync.dma_start(out=xtr[2], in_=xt_d.ap()[1])
            nc.sync.dma_start(out=xtr[3], in_=xt_d.ap()[2])
            nc.sync.dma_start(out=wo, in_=wo_d.ap())

            def xts(ic, qc):
                return xtr[qc][:, ic, :]

            ctxn = ctxnp.tile([128, 2, S], BF16, tag="ctxn")

            # ---- PE warmup: dummy matmuls on a zeroed tile while the first
            # x/w DMAs are in flight, so the tensor engine's clock is ramped
            # when real work arrives.
            wtmp = consts.tile([128, QW], BF16, tag="wtmp")
            nc.vector.memset(wtmp, 0.0)
            wacc = psA.tile([128, 2, QW], F32, tag="st", name="wacc")
            for i in range(16):
                nc.tensor.matmul(
                    wacc[:, 0, 0:192],
                    wtmp[:, 0:128],
                    wtmp[:, 0:192],
                    start=(i == 0),
                    stop=(i == 15),
                )

            # ---- pass1a: qt/kt for qc=0, both pairs, ic-outer (DMA-paced)
            accs = []
            for p in range(2):
                accs.append(psA.tile([128, 2, QW], F32, tag="st", name=f"acc{p}"))
            for half, xh in ((0, xt0a), (1, xt0b)):
                hs = slice(half * (QW // 2), (half + 1) * (QW // 2))
                for ic in range(NIC):
                    for p in range(2):
                        for lane, w in ((0, wq), (1, wk)):
                            nc.tensor.matmul(
                                accs[p][:, lane, hs],
                                w[:, ic, 128 * p : 128 * (p + 1)],
                                xh[:, ic, :],
                                start=(ic == 0),
                                stop=(ic == NIC - 1),
                            )
            for p in range(2):
                nc.any.tensor_copy(qt[:, p, 0:QW], accs[p][:, 0, :])
                nc.any.tensor_copy(kt[0:64, p, 0, 0:QW], accs[p][0:64, 1, :])
                nc.any.tensor_copy(kt[64:128, p, 1, 0:QW], accs[p][64:128, 1, :])

            # ---- filler units (each: PE work from the psO pool + evac)
            def qk_units(dst, w, p, qc):
                # two half-units sharing one accumulator for finer interleave
                hold = {}

                def fa():
                    hold["acc"] = psO.tile([128, QW], F32, tag="po", name="accqk")
                    for ic in range(NIC // 2):
                        nc.tensor.matmul(
                            hold["acc"],
                            w[:, ic, 128 * p : 128 * (p + 1)],
                            xts(ic, qc),
                            start=(ic == 0),
                            stop=False,
                        )

                def fb():
                    acc = hold["acc"]
                    for ic in range(NIC // 2, NIC):
                        nc.tensor.matmul(
                            acc,
                            w[:, ic, 128 * p : 128 * (p + 1)],
                            xts(ic, qc),
                            start=False,
                            stop=(ic == NIC - 1),
                        )
                    qsl = slice(qc * QW, (qc + 1) * QW)
                    if dst is qt:
                        nc.any.tensor_copy(dst[:, p, qsl], acc)
                    else:
                        nc.any.tensor_copy(dst[0:64, p, 0, qsl], acc[0:64, :])
                        nc.any.tensor_copy(dst[64:128, p, 1, qsl], acc[64:128, :])

                return [fa, fb]

            def v_unit(st):
                def f():
                    acc = psO.tile([128, 256], F32, tag="po", name="accv")
                    if st < 4:
                        xh = xt0a if st < 2 else xt0b
                        xs = (st % 2) * 128
                    else:
                        xh = xtr[st // 4]
                        xs = (st % 4) * 128
                    for ic in range(NIC):
                        nc.tensor.matmul(
                            acc,
                            xh[:, ic, xs : xs + 128],
                            wv[:, ic, :],
                            start=(ic == 0),
                            stop=(ic == NIC - 1),
                        )
                    nc.vector.tensor_copy(
                        vext[:, st, :, 0:HD],
                        acc.rearrange("p (h e) -> p h e", h=NHL),
                    )
                    if st < 12:  # st 12-15 are never below the diagonal
                        nc.vector.tensor_copy(
                            vext8[:, st // 2, st % 2, :, 0:HD],
                            acc.rearrange("p (h e) -> p h e", h=NHL),
                        )
                return f

            osb_hold = {}

            def o_unit(t, nh):
                def f():
                    tsl = slice(t * 128, (t + 1) * 128)
                    if nh == 0:
                        osb_hold[t] = outp.tile([128, D], BF16, tag="osb", name="osb")
                    osb = osb_hold[t]
                    po = psO.tile([128, QW], F32, tag="po", name="po")
                    nsl = slice(nh * QW, (nh + 1) * QW)
                    nc.tensor.matmul(
                        po, ctxn[:, 0, tsl], wo[:, 0, nsl], start=True, stop=False
                    )
                    nc.tensor.matmul(
                        po, ctxn[:, 1, tsl], wo[:, 1, nsl], start=False, stop=True
                    )
                    nc.any.tensor_copy(osb[:, nsl], po)
                    if nh == 1:
                        nc.sync.dma_start(out=out_d.ap()[tsl, :], in_=osb)
                return f

            # pair-split out-proj for the last q-chunk: the pair-0 half only
            # needs norm(3,0), so it can fill the ACT-bound attn(3,1) stretch
            stg_hold = {}

            def o_pre(t, nh):
                def f():
                    tsl = slice(t * 128, (t + 1) * 128)
                    po = psO.tile([128, QW], F32, tag="po", name="pop")
                    nsl = slice(nh * QW, (nh + 1) * QW)
                    nc.tensor.matmul(
                        po, ctxn[:, 0, tsl], wo[:, 0, nsl], start=True, stop=True
                    )
                    stg = stgp.tile([128, QW], F32, tag="stg", name="stg")
                    nc.any.tensor_copy(stg, po)
                    stg_hold[(t, nh)] = stg
                return f

            def o_post(t, nh):
                def f():
                    tsl = slice(t * 128, (t + 1) * 128)
                    if nh == 0:
                        osb_hold[t] = outp.tile([128, D], BF16, tag="osb", name="osb")
                    osb = osb_hold[t]
                    po = psO.tile([128, QW], F32, tag="po", name="po")
                    nsl = slice(nh * QW, (nh + 1) * QW)
                    nc.tensor.matmul(
                        po, ctxn[:, 1, tsl], wo[:, 1, nsl], start=True, stop=True
                    )
                    nc.vector.scalar_tensor_tensor(
                        out=osb[:, nsl],
                        in0=po,
                        scalar=1.0,
                        in1=stg_hold[(t, nh)],
                        op0=OP.mult,
                        op1=OP.add,
                    )
                    if nh == 1:
                        nc.sync.dma_start(out=out_d.ap()[tsl, :], in_=osb)
                return f

            Q = []

            def take():
                if Q:
                    Q.pop(0)()

            # ---- attention for one (q-chunk, head-pair): per-kp score tiles
            # with both heads in lanes (row-tiled 64+64), depth-1 ctx pipeline.
            def attn(qc, p):
                qsl = slice(qc * QW, (qc + 1) * QW)
                nkp = 4 * qc + 4
                ctxa = psC.tile([HDP, QW], F32, tag="ctx")
                ctxb = psC.tile([HDP, QW], F32, tag="ctx")
                take()
                take()
                pend = None
                # bulk kp pairs: fp8 DoubleRow ctx (2 k-chunks per matmul)
                for g in range(2 * qc):
                    sta = psA.tile([128, 2, QW], F32, tag="st", name="sta")
                    stb = psA.tile([128, 2, QW], F32, tag="st", name="stb")
                    for kig in range(2):
                        ksl = slice((2 * g + kig) * 128, (2 * g + kig + 1) * 128)
                        for st_t, h in ((sta, 0), (stb, 1)):
                            nc.tensor.matmul(
                                st_t[:, kig, :],
                                kt[:, p, h, ksl],
                                qt[:, p, qsl],
                                start=True,
                                stop=True,
                            )
                    pta = ptp.tile([128, 2, QW], FP8, tag="pt8", name="pta")
                    ptb = ptp.tile([128, 2, QW], FP8, tag="pt8", name="ptb")
                    nc.scalar.activation(pta, sta, AF.Exp, scale=0.125)
                    nc.scalar.activation(ptb, stb, AF.Exp, scale=0.125)
                    if pend is not None:
                        pend()
                    def mkb(g=g, pta=pta, ptb=ptb):
                        def c():
                            for h, ctx_t, pt_t in ((0, ctxa, pta), (1, ctxb, ptb)):
                                nc.tensor.matmul(
                                    ctx_t,
                                    vext8[:, g, :, 2 * p + h, :],
                                    pt_t,
                                    start=(g == 0),
                                    stop=False,
                                    perf_mode=DR,
                                )
                        return c
                    pend = mkb()
                    take()
                    take()
                # diagonal kp chunks: bf16, causally trimmed + masked
                for j in range(4):
                    kp = 4 * qc + j
                    w = QW - 128 * j
                    off = 128 * j
                    ksl = slice(kp * 128, (kp + 1) * 128)
                    qtr = slice(qc * QW + off, (qc + 1) * QW)
                    st = psA.tile([128, 2, QW], F32, tag="st", name="st")
                    pt = ptp.tile([128, 2, QW], BF16, tag="pt", name="pt")
                    for h in range(2):
                        nc.tensor.matmul(
                            st[:, h, 0:w],
                            kt[:, p, h, ksl],
                            qt[:, p, qtr],
                            start=True,
                            stop=True,
                        )
                    nc.scalar.activation(
                        pt[:, :, 0:w], st[:, :, 0:w], AF.Exp, scale=0.125
                    )
                    nc.vector.tensor_mul(
                        pt[:, :, 0:128], pt[:, :, 0:128], maskb[:, :, 0:128]
                    )
                    if pend is not None:
                        pend()
                    def mk(kp=kp, w=w, off=off, pt=pt):
                        def c():
                            for h, ctx_t in ((0, ctxa), (1, ctxb)):
                                nc.tensor.matmul(
                                    ctx_t[0 : HD + 1, off:QW],
                                    vext[:, kp, 2 * p + h, :],
                                    pt[:, h, 0:w],
                                    start=(kp == 0),
                                    stop=(kp == nkp - 1),
                                )
                        return c
                    pend = mk()
                    take()
                pend()
                take()
                take()
                # normalize: ctxn[d, q] = ctx[d, q] * (1 / ctx[64, q])
                for ctx_t, lo in ((ctxa, 0), (ctxb, 64)):
                    dcp = rbp.tile([1, QW], F32, tag="dcp")
                    nc.vector.tensor_copy(dcp, ctx_t[HD : HD + 1, :])
                    rec = rbp.tile([1, QW], F32, tag="rec")
                    nc.vector.reciprocal_approx_fast(rec, dcp)
                    rb = rbp.tile([HD, QW], F32, tag="rb")
                    nc.gpsimd.partition_broadcast(rb, rec)
                    # two halves so the first out-proj tiles unblock earlier
                    for hh in range(2):
                        hsl = slice(hh * 256, (hh + 1) * 256)
                        nc.vector.scalar_tensor_tensor(
                            out=ctxn[lo : lo + HD, p, qc * QW + hh * 256 :
                                     qc * QW + (hh + 1) * 256],
                            in0=ctx_t[0:HD, hsl],
                            scalar=1.0,
                            in1=rb[:, hsl],
                            op0=OP.mult,
                            op1=OP.mult,
                        )

            # ---- schedule
            for st in range(4):
                v_unit(st)()
            for p in range(2):
                Q += qk_units(qt, wq, p, 1)
                Q += qk_units(kt, wk, p, 1)
            attn(0, 0)
            Q += [v_unit(st) for st in range(4, 8)]
            attn(0, 1)
            for p in range(2):
                Q += qk_units(qt, wq, p, 2)
                Q += qk_units(kt, wk, p, 2)
            Q += [o_unit(t, nh) for t in range(0, 3) for nh in range(2)]
            attn(1, 0)
            Q += [v_unit(st) for st in range(8, 12)]
            Q += [o_unit(3, nh) for nh in range(2)]
            attn(1, 1)
            for p in range(2):
                Q += qk_units(qt, wq, p, 3)
                Q += qk_units(kt, wk, p, 3)
            Q += [o_unit(t, nh) for t in range(4, 7) for nh in range(2)]
            attn(2, 0)
            Q += [v_unit(st) for st in range(12, 16)]
            Q += [o_unit(7, nh) for nh in range(2)]
            attn(2, 1)
            Q += [o_unit(t, nh) for t in range(8, 10) for nh in range(2)]
            attn(3, 0)
            Q += [o_pre(t, nh) for t in range(12, 16) for nh in range(2)]
            attn(3, 1)
            # reserved units: fill the PE while norm(3,1) runs on DVE/GpSimd
            for t in range(10, 12):
                for nh in range(2):
                    o_unit(t, nh)()
            while Q:
                take()
            for t in range(12, 16):
                for nh in range(2):
                    o_post(t, nh)()

    nc.finalize()
    return nc


_VONES = np.ones((128, NST * NHL), dtype=ml_dtypes.bfloat16)
_ZZ = np.zeros((128, S), dtype=ml_dtypes.bfloat16)


def _maskb():
    # multiplicative causal mask: 0 where q_local < kp_local, else 1
    m = np.ones((128, QW), dtype=np.float32)
    kp = np.arange(128)[:, None]
    q = np.arange(QW)[None, :]
    m[q < kp] = 0.0
    return np.repeat(m[:, None, :], 2, axis=1).astype(ml_dtypes.bfloat16)


def _pack_w(w):
    # [D, dl] -> [128, D//128, dl] with (p, c, n) = w[128c+p, n]
    d, dl = w.shape
    return np.ascontiguousarray(w.reshape(d // 128, 128, dl).transpose(1, 0, 2))


def shard_inputs(x, Wq, Wk, Wv, Wo):
    x = np.asarray(x, dtype=ml_dtypes.bfloat16)
    # xt[b][r, p, c, s] = x[b, 512r+s, 128c+p]  (matches SBUF tile layout);
    # the first 512 rows are packed as two contiguous 256-row half blocks
    # so the first DMAs land (and projections start) as early as possible.
    xt0s = [
        np.ascontiguousarray(
            x[b][0:QW].reshape(2, QW // 2, NIC, 128).transpose(0, 3, 2, 1)
        )
        for b in range(2)
    ]
    xts = [
        np.ascontiguousarray(
            x[b][QW:].reshape(3, QW, NIC, 128).transpose(0, 3, 2, 1)
        )
        for b in range(2)
    ]
    Wq = np.asarray(Wq, dtype=ml_dtypes.bfloat16)
    Wk = np.asarray(Wk, dtype=ml_dtypes.bfloat16)
    Wv = np.asarray(Wv, dtype=ml_dtypes.bfloat16)
    Wo = np.asarray(Wo, dtype=ml_dtypes.bfloat16)
    mb = _maskb()
    in_maps = []
    for c in range(8):
        b, hg = divmod(c, 4)
        sl = slice(DL * hg, DL * (hg + 1))
        in_maps.append({
            "xt0": xt0s[b],
            "xt": xts[b],
            "wq": _pack_w(Wq[:, sl]),
            "wk": _pack_w(Wk[:, sl]),
            "wv": _pack_w(Wv[:, sl]),
            "wo": _pack_w(Wo[sl, :]),
            "maskb": mb,
            "vones": _VONES,
            "zz": _ZZ,
        })
    return in_maps


def run(inputs, trace=False, **kwargs):
    """Build, run on 8 cores, and return (full_output, BassKernelResults)."""
    nc = build_bass()
    bo = np.asarray(inputs["bo"], dtype=np.float32)
    in_maps = shard_inputs(**{k: v for k, v in inputs.items() if k != "bo"})
    res = run_bass_kernel_spmd(
        nc, in_maps, core_ids=list(range(8)), trace=trace, **kwargs
    )
    parts = [np.asarray(r["out"], dtype=np.float32) for r in res.results]
    out = np.empty((2, S, D), dtype=np.float32)
    for b in range(2):
        out[b] = parts[4 * b] + parts[4 * b + 1] + parts[4 * b + 2] + parts[4 * b + 3]
        out[b] += bo[None, :]
    return out, res


def kernel(x, Wq, Wk, Wv, Wo, bo):
    out, _ = run(dict(x=x, Wq=Wq, Wk=Wk, Wv=Wv, Wo=Wo, bo=bo))
    return out
